# revision 1
# baseline (speedup 1.0000x reference)
"""CrossFocusedLinearAttention Trainium2 kernel.

Per-core computation (1 batch item per NeuronCore, 8 cores):
  q = relu(query @ Wq)/s; k = relu(key_in @ Wk)/s   (s = softplus(scale), folded
  into Wq/Wk columns on host; the +eps inside relu-out is dropped — its relative
  effect is ~1e-6, far below matmul rounding)
  focus(x) = x^3 * ||x|| / ||x^3||  per token (over all C channels)
  per head: kv = k_f^T v ; z = 1/(q_f . ksum + eps); x = (q_f @ kv) * z
  out = x @ Wp + bp

Layout strategy (all contractions on the partition dim, no on-device transposes):
  - host supplies query^T/key_in^T/value^T  [C, N]
  - k, v are produced in natural [token, chan] layout (lhsT = key_in^T blocks)
  - q is produced transposed [chan, token]  (lhsT = Wq blocks)
  - kv accumulated in PSUM over all tokens; the k-side focus renorm rk is
    applied to v instead of k; ksum is computed with rk as the matmul rhs
  - x^T = blockdiag(kv) matmul with q^3 as moving operand; per-(head,
    token) scale g = rq / (rq*t + eps) applied to x^T; out^T = Wp matmul
  - output returned as out^T, transposed back on host

Matmuls run in float32r (TF32-rate: 1 cycle/row at N>=512). Tensors consumed
by matmuls are allocated float32r so producers round on write; non-matmul
readers bitcast back to float32 (same bits).
"""

import os
import sys

import numpy as np

sys.path.insert(0, "/opt/trn_rl_repo")

P = 128
C = 512
N = 4096
CT = C // P            # 4 channel tiles
NH = 8                 # heads
HD = C // NH           # 64 head dim
JBLK = 512             # phase-1 token chunk
JC = N // JBLK         # 8
JSUB = JBLK // P       # 4 token tiles per chunk
IBLK = 512             # phase-2 token chunk
ICN = N // IBLK        # 8
EPS = 1e-6
NCORES = 8

_CACHE = {}


def _tf32_round(a):
    """Round-to-nearest-even fp32 -> tf32 (drop 13 mantissa bits)."""
    i = np.ascontiguousarray(a, np.float32).view(np.uint32)
    lsb = (i >> np.uint32(13)) & np.uint32(1)
    i = (i + np.uint32(0x0FFF) + lsb) & np.uint32(0xFFFFE000)
    return i.view(np.float32)


def _build_nc(mm_dtype_name="float32r"):
    import concourse.mybir as mybir
    import concourse.tile as tile
    from concourse import bacc
    from contextlib import ExitStack

    f32 = mybir.dt.float32
    mdt = getattr(mybir.dt, mm_dtype_name)
    AF = mybir.ActivationFunctionType
    OP = mybir.AluOpType

    def asf32(ap):
        # non-matmul readers of matmul-input tiles: f32r needs a bitcast to
        # plain f32 (same bits); bf16 is read natively by DVE/ACT.
        return ap.bitcast(f32) if mdt == mybir.dt.float32r else ap

    # Pin every ACTIVATE to natural_log_exp_and_others (contains relu,
    # square, ln, exp, identity, copy) — the default greedy set chooser
    # flip-flops between two tables, costing ~1.3us per reload.
    class _BaccOneActTable(bacc.Bacc):
        def insert_act_table_loads(self):
            import bass_rust as _br
            from concourse.hw_specs import get_activation_tables
            has_activation = any(
                isinstance(i, mybir.InstActivation)
                for b in self.main_func.blocks
                for i in b.instructions
            )
            if not has_activation:
                return
            tables = [
                (n, (s if n == "natural_log_exp_and_others" else set()))
                for n, s in get_activation_tables(self.m.arch).items()
            ]
            _br.insert_act_table_loads(self, tables)

    nc = _BaccOneActTable("TRN2", target_bir_lowering=False, debug=False)

    qT = nc.declare_dram_parameter("qT", [C, N], mdt, isOutput=False)
    kT = nc.declare_dram_parameter("kT", [C, N], mdt, isOutput=False)
    vT = nc.declare_dram_parameter("vT", [C, N], mdt, isOutput=False)
    Wq = nc.declare_dram_parameter("Wq", [C, C], mdt, isOutput=False)
    Wk = nc.declare_dram_parameter("Wk", [C, C], mdt, isOutput=False)
    Wv = nc.declare_dram_parameter("Wv", [C, C], mdt, isOutput=False)
    Wp = nc.declare_dram_parameter("Wp", [C, C], mdt, isOutput=False)
    bp_col = nc.declare_dram_parameter("bp_col", [P, CT], f32, isOutput=False)
    sel8 = nc.declare_dram_parameter("sel8", [NH, CT, P], mdt, isOutput=False)
    ones_col = nc.declare_dram_parameter("ones_col", [P, 1], mdt, isOutput=False)
    zeros_in = nc.declare_dram_parameter("zeros_in", [P, P], mdt, isOutput=False)
    outT = nc.declare_dram_parameter("outT", [C, N], f32, isOutput=True)

    # DRAM views: [C, X] -> [P, CT, X] (partition, c-tile, token)
    qT_v = qT.rearrange("(t p) n -> p t n", p=P)
    kT_v = kT.rearrange("(t p) n -> p t n", p=P)
    vT_v = vT.rearrange("(t p) n -> p t n", p=P)
    outT_v = outT.rearrange("(t p) n -> p t n", p=P)
    Wq_v = Wq.rearrange("(t p) n -> p t n", p=P)
    Wk_v = Wk.rearrange("(t p) n -> p t n", p=P)
    Wv_v = Wv.rearrange("(t p) n -> p t n", p=P)
    Wp_v = Wp.rearrange("(t p) n -> p t n", p=P)

    with ExitStack() as ctx:
        tc = ctx.enter_context(tile.TileContext(nc))

        # ---------- persistent SBUF ----------
        wpool = ctx.enter_context(tc.tile_pool(name="weights", bufs=1))
        wk = wpool.tile([P, CT, C], mdt, tag="wk")
        wv = wpool.tile([P, CT, C], mdt, tag="wv")
        wq = wpool.tile([P, CT, C], mdt, tag="wq")
        wp = wpool.tile([P, CT, C], mdt, tag="wp")
        bp_sb = wpool.tile([P, CT], f32, tag="bp")
        ones_sb = wpool.tile([P, 1], mdt, tag="ones")
        sel_sb = wpool.tile([NH, CT, P], mdt, tag="sel8")
        ceps = wpool.tile([P, 1], f32, tag="ceps")
        # phase-1-critical loads only; Wq/Wp and phase-2 constants are
        # loaded mid-phase-1 (below) so they overlap compute instead of
        # delaying the first projection matmuls.
        nc.sync.dma_start(wk[:], Wk_v[:])
        nc.sync.dma_start(wv[:], Wv_v[:])
        nc.sync.dma_start(ones_sb[:], ones_col[:])
        nc.vector.memset(ceps[:], EPS)

        # blockdiag kv + masked ksum (built in transition, used in phase 2)
        bdpool = ctx.enter_context(tc.tile_pool(name="bdkv", bufs=1))
        bd = [bdpool.tile([P, P], mdt, tag=f"bd{t}", name=f"bd{t}")
              for t in range(CT)]
        m8 = [bdpool.tile([P, NH], mdt, tag=f"m8{t}", name=f"m8{t}")
              for t in range(CT)]

        # ================= PHASE 1: k/v -> kv, ksum =================
        with ExitStack() as p1:
            kvpool = p1.enter_context(
                tc.tile_pool(name="kvps", bufs=1, space="PSUM"))
            kv_ab = [kvpool.tile([P, C], f32, tag="kva", name="kva"),
                     kvpool.tile([P, C], f32, tag="kvb", name="kvb")]
            ks_ps = kvpool.tile([P, 2 * CT], f32, tag="ksum")

            kpp = p1.enter_context(
                tc.tile_pool(name="p1kproj", bufs=2, space="PSUM"))
            vpp = p1.enter_context(
                tc.tile_pool(name="p1vproj", bufs=3, space="PSUM"))
            ldp = p1.enter_context(tc.tile_pool(name="p1ld", bufs=2))
            wkp = p1.enter_context(tc.tile_pool(name="p1work", bufs=3))
            smp = p1.enter_context(tc.tile_pool(name="p1small", bufs=4))

            for jc in range(JC):
                ktile = ldp.tile([P, CT, JBLK], mdt, tag="kld")
                nc.sync.dma_start(
                    ktile[:], kT_v[:, :, jc * JBLK:(jc + 1) * JBLK])
                vtile = ldp.tile([P, CT, JBLK], mdt, tag="vld")
                nc.sync.dma_start(
                    vtile[:], vT_v[:, :, jc * JBLK:(jc + 1) * JBLK])
                if jc == 0:
                    nc.sync.dma_start(wq[:], Wq_v[:])
                    nc.sync.dma_start(wp[:], Wp_v[:])
                    nc.sync.dma_start(bp_sb[:], bp_col[:])
                    nc.sync.dma_start(sel_sb[:], sel8[:])

                for jj in range(JSUB):
                    first = (jc == 0 and jj == 0)
                    last = (jc == JC - 1 and jj == JSUB - 1)
                    jsl = slice(jj * P, (jj + 1) * P)

                    kps = kpp.tile([P, C], f32, tag="kproj")
                    for ct in range(CT):
                        nc.tensor.matmul(
                            kps[:], ktile[:, ct, jsl], wk[:, ct, :],
                            start=(ct == 0), stop=(ct == CT - 1))
                    vps = vpp.tile([P, C], f32, tag="vproj")
                    for ct in range(CT):
                        nc.tensor.matmul(
                            vps[:], vtile[:, ct, jsl], wv[:, ct, :],
                            start=(ct == 0), stop=(ct == CT - 1))

                    rlu = wkp.tile([P, C], f32, tag="rlu")
                    nc.scalar.activation(rlu[:], kps[:], AF.Relu)
                    # u2 = rlu^2, S2 = sum_c rlu^2 (per token)
                    u2 = wkp.tile([P, C], f32, tag="u2")
                    S2 = smp.tile([P, 1], f32, tag="s2")
                    nc.scalar.activation(
                        u2[:], rlu[:], AF.Square, accum_out=S2[:])
                    u3 = wkp.tile([P, C], mdt, tag="u3")
                    nc.vector.tensor_tensor(u3[:], u2[:], rlu[:], OP.mult)
                    # u6 scratch + S6 = sum_c u3^2
                    u6 = wkp.tile([P, C], f32, tag="u6")
                    S6 = smp.tile([P, 1], f32, tag="s6")
                    nc.scalar.activation(
                        u6[:], asf32(u3[:]), AF.Square, accum_out=S6[:])
                    # rk = sqrt(S2/S6) = exp(0.5*ln(S2 * (1/S6)))
                    rS6 = smp.tile([P, 1], f32, tag="rs6")
                    nc.vector.reciprocal(rS6[:], S6[:])
                    ratio = smp.tile([P, 1], f32, tag="ratio")
                    nc.vector.tensor_tensor(ratio[:], S2[:], rS6[:], OP.mult)
                    lnr = smp.tile([P, 1], f32, tag="lnr")
                    nc.scalar.activation(lnr[:], ratio[:], AF.Ln)
                    rk = smp.tile([P, 1], f32, tag="rk")
                    nc.scalar.activation(rk[:], lnr[:], AF.Exp, scale=0.5)
                    rkm = smp.tile([P, 2], mdt, tag="rkm")
                    nc.vector.tensor_copy(rkm[:, 0:1], rk[:])
                    nc.vector.tensor_copy(rkm[:, 1:2], rk[:])
                    # v_s = v * rk  (k-side focus renorm folded into v)
                    v_s = wkp.tile([P, C], mdt, tag="vs")
                    nc.vector.tensor_scalar(
                        out=v_s[:], in0=vps[:], scalar1=rk[:],
                        scalar2=None, op0=OP.mult)
                    for ct in range(CT):
                        csl = slice(ct * P, (ct + 1) * P)
                        # kv pair-diag block via a 256-wide window (fp32r
                        # runs full-rate only at N>=256); window = the
                        # 256-col half of v_s containing this pair's cols.
                        wsl = slice((ct // 2) * 256, (ct // 2) * 256 + 256)
                        osl = slice((ct % 2) * 256, (ct % 2) * 256 + 256)
                        nc.tensor.matmul(
                            kv_ab[ct // 2][:, osl], u3[:, csl], v_s[:, wsl],
                            start=(first and ct % 2 == 0),
                            stop=(last and ct % 2 == 1))
                        nc.tensor.matmul(
                            ks_ps[:, 2 * ct:2 * ct + 2], u3[:, csl], rkm[:],
                            start=(first and ct == 0),
                            stop=(last and ct == CT - 1))

            # ---------- transition: blockdiag kv, masked ksum ----------
            for ct in range(CT):
                nc.sync.dma_start(bd[ct][:], zeros_in[:])
                base = (ct % 2) * 384
                nc.vector.tensor_copy(
                    bd[ct][0:HD, 0:HD],
                    kv_ab[ct // 2][0:HD, base:base + HD])
                nc.vector.tensor_copy(
                    bd[ct][HD:P, HD:P],
                    kv_ab[ct // 2][HD:P, base + HD:base + P])
                nc.sync.dma_start(m8[ct][:], zeros_in[:, 0:NH])
                nc.vector.tensor_copy(
                    m8[ct][0:HD, 2 * ct:2 * ct + 1],
                    ks_ps[0:HD, 2 * ct:2 * ct + 1])
                nc.vector.tensor_copy(
                    m8[ct][HD:P, 2 * ct + 1:2 * ct + 2],
                    ks_ps[HD:P, 2 * ct:2 * ct + 1])

        # ================= PHASE 2: q -> x -> out =================
        with ExitStack() as p2:
            qpsp = p2.enter_context(
                tc.tile_pool(name="qps", bufs=2, space="PSUM"))
            stap = p2.enter_context(
                tc.tile_pool(name="stats", bufs=2, space="PSUM"))
            xpsp = p2.enter_context(
                tc.tile_pool(name="xps", bufs=2, space="PSUM"))
            opsp = p2.enter_context(
                tc.tile_pool(name="ops", bufs=2, space="PSUM"))
            ldq = p2.enter_context(tc.tile_pool(name="qld", bufs=2))
            wkq = p2.enter_context(tc.tile_pool(name="p2work", bufs=3))
            u3p = p2.enter_context(tc.tile_pool(name="u3q", bufs=5))
            xsp = p2.enter_context(tc.tile_pool(name="xs", bufs=5))
            osp = p2.enter_context(tc.tile_pool(name="osb", bufs=3))
            smq = p2.enter_context(tc.tile_pool(name="p2small", bufs=3))

            for ic in range(ICN):
                isl = slice(ic * IBLK, (ic + 1) * IBLK)
                qtile = ldq.tile([P, CT, IBLK], mdt, tag="qld")
                nc.sync.dma_start(qtile[:], qT_v[:, :, isl])

                t8 = stap.tile([NH, IBLK], f32, tag="stats")
                xpss = []
                for nt in range(CT):
                    qps = qpsp.tile([P, IBLK], f32, tag="qps")
                    for ct in range(CT):
                        nc.tensor.matmul(
                            qps[:], wq[:, ct, nt * P:(nt + 1) * P],
                            qtile[:, ct, :],
                            start=(ct == 0), stop=(ct == CT - 1))
                    rlu = wkq.tile([P, IBLK], f32, tag="rluq")
                    nc.scalar.activation(rlu[:], qps[:], AF.Relu)
                    u2q = wkq.tile([P, IBLK], f32, tag="u2q")
                    nc.gpsimd.tensor_tensor(u2q[:], rlu[:], rlu[:], OP.mult)
                    u3q = u3p.tile([P, IBLK], mdt, tag="u3q")
                    nc.vector.tensor_tensor(u3q[:], u2q[:], rlu[:], OP.mult)
                    # t8[h] = q^3 . ksum_head  (masked ksum as stationary)
                    nc.tensor.matmul(
                        t8[:], m8[nt][:], u3q[:],
                        start=(nt == 0), stop=(nt == CT - 1))
                    xps = xpsp.tile([P, IBLK], f32, tag="xps")
                    nc.tensor.matmul(
                        xps[:], bd[nt][:], u3q[:], start=True, stop=True)
                    xpss.append(xps)

                # g8 = 1/(t8 + eps) = exp(-ln(t8 + eps))
                # (the q-side focus renorm rq cancels: g = rq/(rq*t+eps)
                #  = 1/(t + eps/rq), and eps/rq ~ eps is ~1e-8 relative)
                lng = smq.tile([NH, IBLK], f32, tag="lng")
                nc.scalar.activation(
                    lng[:], t8[:], AF.Ln, bias=ceps[0:NH, :])
                g8 = smq.tile([NH, IBLK], mdt, tag="g8")
                nc.scalar.activation(g8[:], lng[:], AF.Exp, scale=-1.0)

                for nt in range(CT):
                    gexp_ps = opsp.tile([P, IBLK], f32, tag="ops")
                    nc.tensor.matmul(
                        gexp_ps[:], sel_sb[:, nt, :], g8[:],
                        start=True, stop=True)
                    gexp = wkq.tile([P, IBLK], f32, tag="gexp")
                    nc.vector.tensor_copy(gexp[:], gexp_ps[:])
                    x_s = xsp.tile([P, IBLK], mdt, tag="xs")
                    nc.vector.tensor_tensor(
                        x_s[:], xpss[nt][:], gexp[:], OP.mult)
                    xpss[nt] = x_s

                for et in range(CT):
                    ops_t = opsp.tile([P, IBLK], f32, tag="ops")
                    for nt in range(CT):
                        nc.tensor.matmul(
                            ops_t[:], wp[:, nt, et * P:(et + 1) * P],
                            xpss[nt][:],
                            start=(nt == 0), stop=(nt == CT - 1))
                    out_sb = osp.tile([P, IBLK], f32, tag="osb")
                    nc.scalar.activation(
                        out_sb[:], ops_t[:], AF.Identity,
                        bias=bp_sb[:, et:et + 1])
                    nc.sync.dma_start(outT_v[:, et, isl], out_sb[:])

    nc.compile()
    return nc


def _get_nc():
    key = "nc"
    if key not in _CACHE:
        _CACHE[key] = _build_nc(os.environ.get("CFLA_MM_DTYPE", "float32r"))
    return _CACHE[key]


def _prepare_in_maps(query, key_in, value, Wq, Wk, Wv, Wp, bp, scale):
    query = np.asarray(query, np.float32)
    key_in = np.asarray(key_in, np.float32)
    value = np.asarray(value, np.float32)
    Wq = np.asarray(Wq, np.float32)
    Wk = np.asarray(Wk, np.float32)
    Wv = np.asarray(Wv, np.float32)
    Wp = np.asarray(Wp, np.float32)
    bp = np.asarray(bp, np.float32)
    scale = np.asarray(scale, np.float32)

    B = query.shape[0]
    assert B == NCORES and query.shape[1] == N and query.shape[2] == C

    mmdt = os.environ.get("CFLA_MM_DTYPE", "float32r")
    if mmdt == "bfloat16":
        import ml_dtypes

        def rnd(a):
            return np.ascontiguousarray(
                np.asarray(a, np.float32).astype(ml_dtypes.bfloat16))
    elif mmdt == "float32r":
        def rnd(a):
            return _tf32_round(a)
    else:
        def rnd(a):
            return np.ascontiguousarray(a, np.float32)

    # softplus(scale) folded into Wq/Wk columns (relu(x)/s == relu(x/s), s>0)
    s = np.log1p(np.exp(np.float64(scale.reshape(C)))).astype(np.float32)
    inv_s = (1.0 / s).astype(np.float32)
    Wq_s = rnd(Wq * inv_s[None, :])
    Wk_s = rnd(Wk * inv_s[None, :])
    Wv_r = rnd(Wv)
    Wp_r = rnd(Wp)
    bp_col = np.ascontiguousarray(bp.reshape(CT, P).T)
    ones_col = rnd(np.ones((P, 1), np.float32))
    zeros_in = rnd(np.zeros((P, P), np.float32))
    sel8 = np.zeros((NH, CT, P), np.float32)
    for t in range(CT):
        sel8[2 * t, t, 0:HD] = 1.0
        sel8[2 * t + 1, t, HD:P] = 1.0
    sel8 = rnd(sel8)

    in_maps = []
    for b in range(B):
        in_maps.append({
            "qT": rnd(query[b].T),
            "kT": rnd(key_in[b].T),
            "vT": rnd(value[b].T),
            "Wq": Wq_s, "Wk": Wk_s, "Wv": Wv_r, "Wp": Wp_r,
            "bp_col": bp_col, "sel8": sel8, "ones_col": ones_col, "zeros_in": zeros_in,
        })

    return in_maps


def kernel(query, key_in, value, Wq, Wk, Wv, Wp, bp, scale, H, W):
    from concourse.bass_utils import run_bass_kernel_spmd

    in_maps = _prepare_in_maps(
        query, key_in, value, Wq, Wk, Wv, Wp, bp, scale)
    nc = _get_nc()
    res = run_bass_kernel_spmd(nc, in_maps, list(range(NCORES)))
    out = np.empty((len(in_maps), N, C), np.float32)
    for b in range(len(in_maps)):
        out[b] = res.results[b]["outT"].T
    return out


if __name__ == "__main__":
    rng = np.random.default_rng(0)
    inputs = {
        "query": rng.standard_normal((8, N, C)).astype(np.float32),
        "key_in": rng.standard_normal((8, N, C)).astype(np.float32),
        "value": rng.standard_normal((8, N, C)).astype(np.float32),
        "Wq": (rng.standard_normal((C, C)) * 0.02).astype(np.float32),
        "Wk": (rng.standard_normal((C, C)) * 0.02).astype(np.float32),
        "Wv": (rng.standard_normal((C, C)) * 0.02).astype(np.float32),
        "Wp": (rng.standard_normal((C, C)) * 0.02).astype(np.float32),
        "bp": np.zeros((C,), np.float32),
        "scale": (rng.standard_normal((1, 1, C)) * 0.02).astype(np.float32),
        "H": 64, "W": 64,
    }
    out = kernel(**inputs)
    print("out", out.shape, out.dtype, float(np.abs(out).mean()))

